# revision 11
# baseline (speedup 1.0000x reference)
"""Trainium2 Bass kernel for nn_NodeInfoPropagate (GNN message passing).

Strategy (8 NeuronCores, node-parallel), v2 — pipelined:
  - Shard the 20000 nodes across 8 cores (2500/core, padded to 2560 = 5 tiles
    of 512).  Weights replicated.
  - Activations live on-chip transposed [feature-on-partition, node-on-free];
    all matmuls chain with zero transposes.  fp32 GRU path uses float32r.
  - Per layer the full x table [20000(+pad), 256] bf16 lives in each core's
    HBM; parent + neighbor rows are fetched with dma_gather(transpose=True)
    (SWDGE): cost is ~8ns/idx of GpSimd time, which is the critical resource.
    gather commutes with the linear maps, so only x is gathered and
    summary = x[par] @ Wp.T + mean_k x[nbr] @ Wn.T.
  - v2 overlap fixes vs v1:
      * TWO ping-pong x tables: the AllGather for layer l+1 streams into the
        other buffer while layer l is still gathering from the current one.
      * AllGather is chunked per 512-node tile and issued on the SYNC queue,
        so collectives never block the GpSimd gather queue and overlap with
        tile compute.  Only the last chunk's latency is exposed.
      * outputs written untransposed ([feat, node] f32); host reassembles.
      * per-tile feature loads for layer 0 (no big resident feat buffer).
"""

import sys

sys.path.insert(0, "/opt/trn_rl_repo")

import numpy as np
import ml_dtypes

import concourse.bass as bass
import concourse.bacc as bacc
import concourse.tile as tile
import concourse.mybir as mybir
from concourse import bass_utils

N = 20000
K = 16
H = 256
NCORES = 8
NC_REAL = N // NCORES          # 2500 real nodes per core
NT = 512                       # node tile (matmul free dim / PSUM bank)
T = 5                          # tiles per core
NCP = NT * T                   # 2560 padded nodes per core
ZROW = N                       # all-zero table row for invalid neighbors
NTAB = N + 128                 # table rows
NHALF = (NT // 2) * K          # 4096 neighbor idxs per half-tile
TILE_ROWS = [NT] * (T - 1) + [NC_REAL - NT * (T - 1)]  # [512,512,512,512,452]
# chunk-major table layout: chunk t holds all 8 cores' tile-t rows
# contiguously, so each AllGather chunk writes a contiguous range.
CHUNK_OFF = [0]
for _r in TILE_ROWS:
    CHUNK_OFF.append(CHUNK_OFF[-1] + NCORES * _r)  # [0,4096,...,16384,20000]

F32 = mybir.dt.float32
F32R = mybir.dt.float32r
BF16 = mybir.dt.bfloat16
I16 = mybir.dt.int16
BF = ml_dtypes.bfloat16

_CACHE = {}


def _build(depth: int):
    nc = bacc.Bacc("TRN2", target_bir_lowering=False, debug=False,
                   num_devices=NCORES)

    featT = nc.dram_tensor("featT", [128, 2, NCP], F32, kind="ExternalInput")
    invcnt = nc.dram_tensor("invcnt", [128, NCP], F32, kind="ExternalInput")
    # per tile: 512 cols of wrapped neighbor idxs (8192), then 32 cols of
    # wrapped parent idxs (512) -- gathered as two instructions (8192 is the
    # max num_idxs a single SWDGE gather supports)
    gidx = nc.dram_tensor("gidx", [128, T, NT + NT // 16], I16, kind="ExternalInput")
    w_in = nc.dram_tensor("w_in", [128, 2, H], F32, kind="ExternalInput")
    w_ih = nc.dram_tensor("w_ih", [128, 2, 3 * H], F32, kind="ExternalInput")
    w_hh = nc.dram_tensor("w_hh", [128, 2, 3 * H], F32, kind="ExternalInput")
    w_p = nc.dram_tensor("w_p", [128, 2, H], BF16, kind="ExternalInput")
    w_n = nc.dram_tensor("w_n", [128, 2, H], BF16, kind="ExternalInput")
    # bias columns: 0-1 b_in, 2-3 b_p+b_n, 4-5 b_r, 6-7 b_z, 8-9 b_ih_n,
    # 10-11 b_hh_n  (per 128-feature chunk)
    biases = nc.dram_tensor("biases", [128, 12], F32, kind="ExternalInput")
    ident_b = nc.dram_tensor("ident_b", [128, 128], BF16, kind="ExternalInput")
    # output transposed: y[p, ch, i] = x_out[node i][ch*128 + p]
    y = nc.dram_tensor("y", [128, 2, NCP], F32, kind="ExternalOutput")

    SIG = mybir.ActivationFunctionType.Sigmoid
    TANH = mybir.ActivationFunctionType.Tanh
    ADD = mybir.AluOpType.add
    MULT = mybir.AluOpType.mult

    with tile.TileContext(nc) as tc:
        with (
            tc.tile_pool(name="const", bufs=1) as constp,
            tc.tile_pool(name="state", bufs=1) as statep,
            tc.tile_pool(name="dram", bufs=1, space="DRAM") as dramp,
            tc.tile_pool(name="feat", bufs=2) as featp,
            tc.tile_pool(name="gath", bufs=4) as gathp,
            tc.tile_pool(name="pgath", bufs=3) as pgathp,
            tc.tile_pool(name="work", bufs=2) as workp,
            tc.tile_pool(name="tmp", bufs=2) as tmpp,
            tc.tile_pool(name="ps", bufs=2, space="PSUM") as psp,
            tc.tile_pool(name="psg", bufs=6, space="PSUM") as psgp,
        ):
            # ---- resident constants -------------------------------------
            win_sb = constp.tile([128, 2, H], F32R, name="win_sb")
            nc.sync.dma_start(win_sb[:], w_in.ap().bitcast(F32R))
            wih_sb = constp.tile([128, 2, 3 * H], F32R, name="wih_sb")
            nc.sync.dma_start(wih_sb[:], w_ih.ap().bitcast(F32R))
            whh_sb = constp.tile([128, 2, 3 * H], F32R, name="whh_sb")
            nc.sync.dma_start(whh_sb[:], w_hh.ap().bitcast(F32R))
            wp_sb = constp.tile([128, 2, H], BF16, name="wp_sb")
            nc.sync.dma_start(wp_sb[:], w_p.ap())
            wn_sb = constp.tile([128, 2, H], BF16, name="wn_sb")
            nc.sync.dma_start(wn_sb[:], w_n.ap())
            bias_sb = constp.tile([128, 12], F32, name="bias_sb")
            nc.sync.dma_start(bias_sb[:], biases.ap())
            idb_sb = constp.tile([128, 128], BF16, name="idb_sb")
            nc.sync.dma_start(idb_sb[:], ident_b.ap())
            inv_sb = constp.tile([128, NCP], F32, name="inv_sb")
            nc.sync.dma_start(inv_sb[:], invcnt.ap())
            gidx_sb = constp.tile([128, T, NT + NT // 16], I16, name="gidx_sb")
            nc.sync.dma_start(gidx_sb[:], gidx.ap())

            xF = [statep.tile([128, 2, NCP], F32R, name=f"xF{i}") for i in range(2)]

            xloc = dramp.tile([NCP, H], BF16, name="xloc")
            xtabs = [dramp.tile([NTAB, H], BF16, name=f"xtab{i}") for i in range(2)]

            # zero row for invalid-neighbor gathers (both tables, once)
            zero_sb = constp.tile([128, H], BF16, name="zero_sb")
            nc.vector.memset(zero_sb[:], 0.0)
            for xt in xtabs:
                nc.sync.dma_start(xt[ZROW:ZROW + 1, :], zero_sb[0:1, :])

            def write_table_tile(xf, t):
                """cast tile t of xf to bf16, transpose to row-major, DMA to
                xloc rows."""
                ts = slice(t * NT, (t + 1) * NT)
                xb = workp.tile([128, 2, NT], BF16, tag="xb", name="xb")
                nc.vector.tensor_copy(xb[:], xf[:, :, ts].bitcast(F32))
                for b in range(NT // 128):
                    rm = workp.tile([128, 2, 128], BF16, tag="rm", name="rm")
                    for c in range(2):
                        pst = psp.tile([128, 128], BF16, tag="sum", name="pst")
                        nc.tensor.transpose(pst[:], xb[:, c, b * 128:(b + 1) * 128],
                                            idb_sb[:])
                        nc.vector.tensor_copy(rm[:, c, :], pst[:])
                    r0 = t * NT + b * 128
                    nc.sync.dma_start(xloc[r0:r0 + 128, :], rm[:])

            def ag_chunk(t, xtab_dst):
                """AllGather tile-t rows of xloc into the strided per-core
                slots of xtab_dst.  Issued on the SYNC queue so it never
                blocks the gather (GpSimd) queue."""
                rows = TILE_ROWS[t]
                r0 = t * NT
                ins_ap = xloc[r0:r0 + rows, :]
                o0 = CHUNK_OFF[t]
                outs_ap = xtab_dst[o0:o0 + NCORES * rows, :]
                with tc.high_priority():
                    bass.BassGpSimd.collective_compute(
                        nc.gpsimd,
                        "AllGather", mybir.AluOpType.bypass,
                        replica_groups=[list(range(NCORES))],
                        ins=[ins_ap.opt()],
                        outs=[outs_ap.opt()],
                    )

            # ---- layer 0: x0 = W_in @ feat + b_in ------------------------
            for t in range(T):
                ts = slice(t * NT, (t + 1) * NT)
                ft = featp.tile([128, 2, NT], F32R, tag="ft", name="ft")
                nc.sync.dma_start(ft[:], featT.ap().bitcast(F32R)[:, :, ts])
                for oc in range(2):
                    ps = psp.tile([128, NT], F32, tag="sum", name="ps0")
                    for dc in range(2):
                        nc.tensor.matmul(ps[:], win_sb[:, dc, oc * 128:(oc + 1) * 128],
                                         ft[:, dc, :], start=(dc == 0), stop=(dc == 1))
                    nc.vector.tensor_scalar_add(xF[0][:, oc, ts], ps[:],
                                                bias_sb[:, oc:oc + 1])
                if depth == 0:
                    nc.sync.dma_start(y.ap()[:, :, ts],
                                      xF[0][:, :, ts].bitcast(F32))
                else:
                    write_table_tile(xF[0], t)
                    ag_chunk(t, xtabs[0])

            # ---- GRU layers ---------------------------------------------
            cur = 0
            for layer in range(depth):
                LBASE = 1.0 + layer * 1.0
                last = layer == depth - 1
                xf_in, xf_out = xF[cur], xF[1 - cur]
                xtab_in = xtabs[layer % 2]
                xtab_out = xtabs[(layer + 1) % 2]
                for t in range(T):
                    ts = slice(t * NT, (t + 1) * NT)
                    # transpose-gathers cap at 4096 idxs: two neighbor halves
                    pg = pgathp.tile([128, 2, NT], BF16, tag="pgat", name="pg")
                    with tc.tile_wait_until(LBASE + t * 0.1):
                        nc.gpsimd.dma_gather(pg[:], xtab_in[:],
                                             gidx_sb[:, t, NT:],
                                             NT, NT, H, transpose=True,
                                             single_packet=False)
                    pgat = pg[:, :, :]
                    nsum = workp.tile([128, 2, NT], F32, tag="nsum", name="nsum")
                    for hf in range(2):
                        hs = slice(hf * (NT // 2), (hf + 1) * (NT // 2))
                        ngat = gathp.tile([128, 2, NHALF], BF16, tag="ngat",
                                          name="ngat")
                        with tc.tile_wait_until(LBASE + t * 0.1 + 0.02 + hf * 0.03):
                            nc.gpsimd.dma_gather(
                                ngat[:], xtab_in[:],
                                gidx_sb[:, t, hf * (NT // 2):(hf + 1) * (NT // 2)],
                                NHALF, NHALF, H, transpose=True,
                                single_packet=False)
                        for c in range(2):
                            nc.vector.tensor_reduce(
                                nsum[:, c, hs],
                                ngat[:, c, :].rearrange("p (n k) -> p n k", k=K),
                                axis=mybir.AxisListType.X, op=ADD)
                    if not last and t >= 1:
                        with tc.tile_wait_until(LBASE + t * 0.1 + 0.07):
                            ag_chunk(t - 1, xtab_out)
                    nmean = workp.tile([128, 2, NT], BF16, tag="nmean", name="nmean")
                    for c in range(2):
                        nc.vector.tensor_mul(nmean[:, c, :], nsum[:, c, :],
                                             inv_sb[:, ts])
                    # summary = pgat @ Wp.T + nmean @ Wn.T + (b_p + b_n)
                    sT = workp.tile([128, 2, NT], F32R, tag="sT", name="sT")
                    for oc in range(2):
                        ps = psp.tile([128, NT], F32, tag="sum", name="psS")
                        for hc in range(2):
                            nc.tensor.matmul(ps[:],
                                             wp_sb[:, hc, oc * 128:(oc + 1) * 128],
                                             pgat[:, hc, :],
                                             start=(hc == 0), stop=False)
                        for hc in range(2):
                            nc.tensor.matmul(ps[:],
                                             wn_sb[:, hc, oc * 128:(oc + 1) * 128],
                                             nmean[:, hc, :],
                                             start=False, stop=(hc == 1))
                        nc.vector.tensor_scalar_add(sT[:, oc, :], ps[:],
                                                    bias_sb[:, 2 + oc:3 + oc])
                    # GRU gates, per output chunk
                    for oc in range(2):
                        rp = psgp.tile([128, NT], F32, tag="gate", name="rp")
                        zp = psgp.tile([128, NT], F32, tag="gate", name="zp")
                        ip = psgp.tile([128, NT], F32, tag="gate", name="ip")
                        hp = psgp.tile([128, NT], F32, tag="gate", name="hp")
                        for gate, pst in ((0, rp), (1, zp)):
                            o0 = gate * H + oc * 128
                            for hc in range(2):
                                nc.tensor.matmul(pst[:], wih_sb[:, hc, o0:o0 + 128],
                                                 xf_in[:, hc, ts],
                                                 start=(hc == 0), stop=False)
                            for hc in range(2):
                                nc.tensor.matmul(pst[:], whh_sb[:, hc, o0:o0 + 128],
                                                 sT[:, hc, :],
                                                 start=False, stop=(hc == 1))
                        o0 = 2 * H + oc * 128
                        for hc in range(2):
                            nc.tensor.matmul(ip[:], wih_sb[:, hc, o0:o0 + 128],
                                             xf_in[:, hc, ts],
                                             start=(hc == 0), stop=(hc == 1))
                        for hc in range(2):
                            nc.tensor.matmul(hp[:], whh_sb[:, hc, o0:o0 + 128],
                                             sT[:, hc, :],
                                             start=(hc == 0), stop=(hc == 1))
                        r = tmpp.tile([128, NT], F32, tag="r", name="r")
                        nc.scalar.activation(r[:], rp[:], SIG,
                                             bias=bias_sb[:, 4 + oc:5 + oc])
                        z = tmpp.tile([128, NT], F32, tag="z", name="z")
                        nc.scalar.activation(z[:], zp[:], SIG,
                                             bias=bias_sb[:, 6 + oc:7 + oc])
                        # n = tanh((i_n + b_ih_n) + r * (h_n + b_hh_n))
                        hnr = tmpp.tile([128, NT], F32, tag="hnr", name="hnr")
                        nc.vector.scalar_tensor_tensor(
                            hnr[:], hp[:], bias_sb[:, 10 + oc:11 + oc], r[:],
                            op0=ADD, op1=MULT)
                        npre = tmpp.tile([128, NT], F32, tag="npre", name="npre")
                        nc.vector.scalar_tensor_tensor(
                            npre[:], ip[:], bias_sb[:, 8 + oc:9 + oc], hnr[:],
                            op0=ADD, op1=ADD)
                        nt_ = tmpp.tile([128, NT], F32, tag="nt", name="nt")
                        nc.scalar.activation(nt_[:], npre[:], TANH)
                        # x_new = n + z * (summary - n)
                        d = tmpp.tile([128, NT], F32, tag="d", name="d")
                        nc.vector.tensor_sub(d[:], sT[:, oc, :].bitcast(F32), nt_[:])
                        dz = tmpp.tile([128, NT], F32, tag="dz", name="dz")
                        nc.vector.tensor_mul(dz[:], d[:], z[:])
                        nc.vector.tensor_add(xf_out[:, oc, ts], dz[:], nt_[:])
                    if last:
                        nc.sync.dma_start(y.ap()[:, :, ts],
                                          xf_out[:, :, ts].bitcast(F32))
                    else:
                        write_table_tile(xf_out, t)
                if not last:
                    with tc.tile_wait_until(LBASE + T * 0.1 + 0.05):
                        ag_chunk(T - 1, xtab_out)
                cur = 1 - cur

    nc.compile()
    return nc


def _get_nc(depth: int):
    if depth not in _CACHE:
        _CACHE[depth] = _build(depth)
    return _CACHE[depth]


def _idx_layout(lin):
    """linear int16 idx list (len % 16 == 0) -> [128, len//16] wrapped in 16
    partitions, replicated across the 8 gpsimd core groups."""
    v = lin.reshape(-1, 16).T.astype(np.int16)        # [16, len//16]
    return np.tile(v, (8, 1))                         # [128, len//16]


def _chunk2(w):
    """[256, M] -> [128, 2, M] with [p, c, m] = w[c*128+p, m]."""
    M = w.shape[1]
    return np.ascontiguousarray(w.reshape(2, 128, M).transpose(1, 0, 2))


def prepare_inputs(inputs):
    """host-side preprocessing: returns in_maps for the 8 cores."""
    adj = np.asarray(inputs["nodeAdjacencySpecTensor"]).astype(np.int64)
    names = np.asarray(inputs["nodeNamesEncoded"], dtype=np.float32)
    attrs = np.asarray(inputs["nodeAttributesEncoded"], dtype=np.float32)

    parent = adj[:, 0]
    parent = np.clip(np.where(parent < 0, parent + N, parent), 0, N - 1)
    nbr = adj[:, 1:]
    mask = nbr >= 0
    cnt = np.maximum(mask.sum(1), 1).astype(np.float32)
    safe = np.where(mask, np.clip(nbr, 0, N - 1), ZROW).astype(np.int64)

    # remap global node id -> chunk-major xtab row (ZROW maps to itself)
    g = np.arange(N, dtype=np.int64)
    c_of = g // NC_REAL
    r_of = g % NC_REAL
    t_of = np.minimum(r_of // NT, T - 1)
    rows_t = np.asarray(TILE_ROWS, np.int64)[t_of]
    off_t = np.asarray(CHUNK_OFF[:T], np.int64)[t_of]
    remap = np.concatenate([off_t + c_of * rows_t + (r_of - t_of * NT),
                            np.array([ZROW], np.int64)])
    parent = remap[parent]
    safe = remap[safe]
    inv = (1.0 / cnt).astype(np.float32)

    feat = np.concatenate([names, attrs], axis=1)      # [N, 256] f32

    W_in = np.asarray(inputs["W_in"], np.float32)
    W_p = np.asarray(inputs["W_parent"], np.float32)
    W_n = np.asarray(inputs["W_neighbor"], np.float32)
    W_ih = np.asarray(inputs["W_ih"], np.float32)
    W_hh = np.asarray(inputs["W_hh"], np.float32)
    b_in = np.asarray(inputs["b_in"], np.float32)
    b_p = np.asarray(inputs["b_parent"], np.float32)
    b_n = np.asarray(inputs["b_neighbor"], np.float32)
    b_ih = np.asarray(inputs["b_ih"], np.float32)
    b_hh = np.asarray(inputs["b_hh"], np.float32)

    w_in_a = _chunk2(W_in.T)                            # [128, 2, 256]
    w_ih_a = _chunk2(W_ih.T)                            # [128, 2, 768]
    w_hh_a = _chunk2(W_hh.T)
    w_p_a = _chunk2(W_p.T).astype(BF)
    w_n_a = _chunk2(W_n.T).astype(BF)

    bias = np.zeros((128, 12), np.float32)
    for col, vec in ((0, b_in), (2, b_p + b_n), (4, (b_ih + b_hh)[0:H]),
                     (6, (b_ih + b_hh)[H:2 * H]), (8, b_ih[2 * H:3 * H]),
                     (10, b_hh[2 * H:3 * H])):
        bias[:, col] = vec[0:128]
        bias[:, col + 1] = vec[128:256]

    ident_b = np.eye(128, dtype=BF)

    shared = dict(w_in=w_in_a, w_ih=w_ih_a, w_hh=w_hh_a, w_p=w_p_a, w_n=w_n_a,
                  biases=bias, ident_b=ident_b)

    in_maps = []
    for c in range(NCORES):
        g0 = c * NC_REAL
        # features, transposed + padded
        f = np.zeros((NCP, 2 * 128), np.float32)
        f[:NC_REAL] = feat[g0:g0 + NC_REAL]
        featT_c = np.ascontiguousarray(
            f.T.reshape(2, 128, NCP).transpose(1, 0, 2))
        # inv count broadcast
        iv = np.ones(NCP, np.float32)
        iv[:NC_REAL] = inv[g0:g0 + NC_REAL]
        inv_c = np.broadcast_to(iv, (128, NCP)).copy()
        # indices
        par = np.full(NCP, ZROW, np.int64)
        par[:NC_REAL] = parent[g0:g0 + NC_REAL]
        nbrs = np.full((NCP, K), ZROW, np.int64)
        nbrs[:NC_REAL] = safe[g0:g0 + NC_REAL]
        gidx_t = np.zeros((128, T, NT + NT // 16), np.int16)
        for t in range(T):
            lin = np.concatenate([nbrs[t * NT:(t + 1) * NT].reshape(-1),
                                  par[t * NT:(t + 1) * NT]])
            gidx_t[:, t, :] = _idx_layout(lin)
        in_maps.append(dict(featT=featT_c, invcnt=inv_c, gidx=gidx_t,
                            **shared))
    return in_maps


def run(inputs, trace=False, **kw):
    depth = int(np.asarray(inputs["depth"]))
    nc = _get_nc(depth)
    in_maps = prepare_inputs(inputs)
    res = bass_utils.run_bass_kernel_spmd(nc, in_maps,
                                          core_ids=list(range(NCORES)),
                                          trace=trace, **kw)
    outs = []
    for c in range(NCORES):
        yt = np.asarray(res.results[c]["y"])          # [128, 2, NCP]
        xc = yt.transpose(1, 0, 2).reshape(2 * 128, NCP)   # [256, NCP]
        outs.append(xc[:, :NC_REAL].T)                # [NC_REAL, 256]
    out = np.concatenate(outs, axis=0)
    return np.ascontiguousarray(out.astype(np.float32)), res


def kernel(**inputs) -> np.ndarray:
    out, _ = run(inputs, trace=False)
    return out


# revision 12
# speedup vs baseline: 1.0165x; 1.0165x over previous
"""Trainium2 Bass kernel for nn_NodeInfoPropagate (GNN message passing).

Strategy (8 NeuronCores, node-parallel), v2 — pipelined:
  - Shard the 20000 nodes across 8 cores (2500/core, padded to 2560 = 5 tiles
    of 512).  Weights replicated.
  - Activations live on-chip transposed [feature-on-partition, node-on-free];
    all matmuls chain with zero transposes.  fp32 GRU path uses float32r.
  - Per layer the full x table [20000(+pad), 256] bf16 lives in each core's
    HBM; parent + neighbor rows are fetched with dma_gather(transpose=True)
    (SWDGE): cost is ~8ns/idx of GpSimd time, which is the critical resource.
    gather commutes with the linear maps, so only x is gathered and
    summary = x[par] @ Wp.T + mean_k x[nbr] @ Wn.T.
  - v2 overlap fixes vs v1:
      * TWO ping-pong x tables: the AllGather for layer l+1 streams into the
        other buffer while layer l is still gathering from the current one.
      * AllGather is chunked per 512-node tile and issued on the SYNC queue,
        so collectives never block the GpSimd gather queue and overlap with
        tile compute.  Only the last chunk's latency is exposed.
      * outputs written untransposed ([feat, node] f32); host reassembles.
      * per-tile feature loads for layer 0 (no big resident feat buffer).
"""

import sys

sys.path.insert(0, "/opt/trn_rl_repo")

import numpy as np
import ml_dtypes

import concourse.bass as bass
import concourse.bacc as bacc
import concourse.tile as tile
import concourse.mybir as mybir
from concourse import bass_utils

N = 20000
K = 16
H = 256
NCORES = 8
NC_REAL = N // NCORES          # 2500 real nodes per core
NT = 512                       # node tile (matmul free dim / PSUM bank)
T = 5                          # tiles per core
NCP = NT * T                   # 2560 padded nodes per core
ZROW = N                       # all-zero table row for invalid neighbors
NTAB = N + 128                 # table rows
NHALF = (NT // 2) * K          # 4096 neighbor idxs per half-tile
TILE_ROWS = [NT] * (T - 1) + [NC_REAL - NT * (T - 1)]  # [512,512,512,512,452]
# chunk-major table layout: chunk t holds all 8 cores' tile-t rows
# contiguously, so each AllGather chunk writes a contiguous range.
CHUNK_OFF = [0]
for _r in TILE_ROWS:
    CHUNK_OFF.append(CHUNK_OFF[-1] + NCORES * _r)  # [0,4096,...,16384,20000]

F32 = mybir.dt.float32
F32R = mybir.dt.float32r
BF16 = mybir.dt.bfloat16
I16 = mybir.dt.int16
BF = ml_dtypes.bfloat16

_CACHE = {}


def _build(depth: int):
    nc = bacc.Bacc("TRN2", target_bir_lowering=False, debug=False,
                   num_devices=NCORES)

    featT = nc.dram_tensor("featT", [128, 2, NCP], F32, kind="ExternalInput")
    invcnt = nc.dram_tensor("invcnt", [128, NCP], F32, kind="ExternalInput")
    # per tile: 512 cols of wrapped neighbor idxs (8192), then 32 cols of
    # wrapped parent idxs (512) -- gathered as two instructions (8192 is the
    # max num_idxs a single SWDGE gather supports)
    gidx = nc.dram_tensor("gidx", [128, T, NT + NT // 16], I16, kind="ExternalInput")
    w_in = nc.dram_tensor("w_in", [128, 2, H], F32, kind="ExternalInput")
    w_ih = nc.dram_tensor("w_ih", [128, 2, 3 * H], F32, kind="ExternalInput")
    w_hh = nc.dram_tensor("w_hh", [128, 2, 3 * H], F32, kind="ExternalInput")
    w_p = nc.dram_tensor("w_p", [128, 2, H], BF16, kind="ExternalInput")
    w_n = nc.dram_tensor("w_n", [128, 2, H], BF16, kind="ExternalInput")
    # bias columns: 0-1 b_in, 2-3 b_p+b_n, 4-5 b_r, 6-7 b_z, 8-9 b_ih_n,
    # 10-11 b_hh_n  (per 128-feature chunk)
    biases = nc.dram_tensor("biases", [128, 12], F32, kind="ExternalInput")
    ident_b = nc.dram_tensor("ident_b", [128, 128], BF16, kind="ExternalInput")
    # output transposed: y[p, ch, i] = x_out[node i][ch*128 + p]
    y = nc.dram_tensor("y", [128, 2, NCP], F32, kind="ExternalOutput")

    SIG = mybir.ActivationFunctionType.Sigmoid
    TANH = mybir.ActivationFunctionType.Tanh
    ADD = mybir.AluOpType.add
    MULT = mybir.AluOpType.mult

    with tile.TileContext(nc) as tc:
        with (
            tc.tile_pool(name="const", bufs=1) as constp,
            tc.tile_pool(name="state", bufs=1) as statep,
            tc.tile_pool(name="dram", bufs=1, space="DRAM") as dramp,
            tc.tile_pool(name="feat", bufs=2) as featp,
            tc.tile_pool(name="gath", bufs=4) as gathp,
            tc.tile_pool(name="pgath", bufs=4) as pgathp,
            tc.tile_pool(name="work", bufs=2) as workp,
            tc.tile_pool(name="tmp", bufs=2) as tmpp,
            tc.tile_pool(name="ps", bufs=2, space="PSUM") as psp,
            tc.tile_pool(name="psg", bufs=6, space="PSUM") as psgp,
        ):
            # ---- resident constants -------------------------------------
            win_sb = constp.tile([128, 2, H], F32R, name="win_sb")
            nc.sync.dma_start(win_sb[:], w_in.ap().bitcast(F32R))
            wih_sb = constp.tile([128, 2, 3 * H], F32R, name="wih_sb")
            nc.sync.dma_start(wih_sb[:], w_ih.ap().bitcast(F32R))
            whh_sb = constp.tile([128, 2, 3 * H], F32R, name="whh_sb")
            nc.sync.dma_start(whh_sb[:], w_hh.ap().bitcast(F32R))
            wp_sb = constp.tile([128, 2, H], BF16, name="wp_sb")
            nc.sync.dma_start(wp_sb[:], w_p.ap())
            wn_sb = constp.tile([128, 2, H], BF16, name="wn_sb")
            nc.sync.dma_start(wn_sb[:], w_n.ap())
            bias_sb = constp.tile([128, 12], F32, name="bias_sb")
            nc.sync.dma_start(bias_sb[:], biases.ap())
            idb_sb = constp.tile([128, 128], BF16, name="idb_sb")
            nc.sync.dma_start(idb_sb[:], ident_b.ap())
            inv_sb = constp.tile([128, NCP], F32, name="inv_sb")
            nc.sync.dma_start(inv_sb[:], invcnt.ap())
            gidx_sb = constp.tile([128, T, NT + NT // 16], I16, name="gidx_sb")
            nc.sync.dma_start(gidx_sb[:], gidx.ap())

            xF = [statep.tile([128, 2, NCP], F32R, name=f"xF{i}") for i in range(2)]

            xloc = dramp.tile([NCP, H], BF16, name="xloc")
            xtabs = [dramp.tile([NTAB, H], BF16, name=f"xtab{i}") for i in range(2)]

            # zero row for invalid-neighbor gathers (both tables, once)
            zero_sb = constp.tile([128, H], BF16, name="zero_sb")
            nc.vector.memset(zero_sb[:], 0.0)
            for xt in xtabs:
                nc.sync.dma_start(xt[ZROW:ZROW + 1, :], zero_sb[0:1, :])

            def write_table_tile(xf, t):
                """cast tile t of xf to bf16, transpose to row-major, DMA to
                xloc rows."""
                ts = slice(t * NT, (t + 1) * NT)
                xb = workp.tile([128, 2, NT], BF16, tag="xb", name="xb")
                nc.vector.tensor_copy(xb[:], xf[:, :, ts].bitcast(F32))
                for b in range(NT // 128):
                    rm = workp.tile([128, 2, 128], BF16, tag="rm", name="rm")
                    for c in range(2):
                        pst = psp.tile([128, 128], BF16, tag="sum", name="pst")
                        nc.tensor.transpose(pst[:], xb[:, c, b * 128:(b + 1) * 128],
                                            idb_sb[:])
                        nc.vector.tensor_copy(rm[:, c, :], pst[:])
                    r0 = t * NT + b * 128
                    nc.sync.dma_start(xloc[r0:r0 + 128, :], rm[:])

            def ag_chunk(t, xtab_dst):
                """AllGather tile-t rows of xloc into the strided per-core
                slots of xtab_dst.  Issued on the SYNC queue so it never
                blocks the gather (GpSimd) queue."""
                rows = TILE_ROWS[t]
                r0 = t * NT
                ins_ap = xloc[r0:r0 + rows, :]
                o0 = CHUNK_OFF[t]
                outs_ap = xtab_dst[o0:o0 + NCORES * rows, :]
                with tc.high_priority():
                    bass.BassGpSimd.collective_compute(
                        nc.gpsimd,
                        "AllGather", mybir.AluOpType.bypass,
                        replica_groups=[list(range(NCORES))],
                        ins=[ins_ap.opt()],
                        outs=[outs_ap.opt()],
                    )

            # ---- layer 0: x0 = W_in @ feat + b_in ------------------------
            for t in range(T):
                ts = slice(t * NT, (t + 1) * NT)
                ft = featp.tile([128, 2, NT], F32R, tag="ft", name="ft")
                nc.sync.dma_start(ft[:], featT.ap().bitcast(F32R)[:, :, ts])
                for oc in range(2):
                    ps = psp.tile([128, NT], F32, tag="sum", name="ps0")
                    for dc in range(2):
                        nc.tensor.matmul(ps[:], win_sb[:, dc, oc * 128:(oc + 1) * 128],
                                         ft[:, dc, :], start=(dc == 0), stop=(dc == 1))
                    nc.vector.tensor_scalar_add(xF[0][:, oc, ts], ps[:],
                                                bias_sb[:, oc:oc + 1])
                if depth == 0:
                    nc.sync.dma_start(y.ap()[:, :, ts],
                                      xF[0][:, :, ts].bitcast(F32))
                else:
                    write_table_tile(xF[0], t)
                    ag_chunk(t, xtabs[0])

            # ---- GRU layers ---------------------------------------------
            cur = 0
            for layer in range(depth):
                LBASE = 1.0 + layer * 1.0
                last = layer == depth - 1
                xf_in, xf_out = xF[cur], xF[1 - cur]
                xtab_in = xtabs[layer % 2]
                xtab_out = xtabs[(layer + 1) % 2]
                for t in range(T):
                    ts = slice(t * NT, (t + 1) * NT)
                    # transpose-gathers cap at 4096 idxs: two neighbor halves
                    pg = pgathp.tile([128, 2, NT], BF16, tag="pgat", name="pg")
                    with tc.tile_wait_until(LBASE + t * 0.1):
                        nc.gpsimd.dma_gather(pg[:], xtab_in[:],
                                             gidx_sb[:, t, NT:],
                                             NT, NT, H, transpose=True,
                                             single_packet=False)
                    pgat = pg[:, :, :]
                    nsum = workp.tile([128, 2, NT], F32, tag="nsum", name="nsum")
                    for hf in range(2):
                        hs = slice(hf * (NT // 2), (hf + 1) * (NT // 2))
                        ngat = gathp.tile([128, 2, NHALF], BF16, tag="ngat",
                                          name="ngat")
                        with tc.tile_wait_until(LBASE + t * 0.1 + 0.02 + hf * 0.03):
                            nc.gpsimd.dma_gather(
                                ngat[:], xtab_in[:],
                                gidx_sb[:, t, hf * (NT // 2):(hf + 1) * (NT // 2)],
                                NHALF, NHALF, H, transpose=True,
                                single_packet=False)
                        with tc.high_priority():
                            for c in range(2):
                                nc.vector.tensor_reduce(
                                    nsum[:, c, hs],
                                    ngat[:, c, :].rearrange("p (n k) -> p n k", k=K),
                                    axis=mybir.AxisListType.X, op=ADD)
                    if not last and t >= 1:
                        with tc.tile_wait_until(LBASE + t * 0.1 + 0.07):
                            ag_chunk(t - 1, xtab_out)
                    nmean = workp.tile([128, 2, NT], BF16, tag="nmean", name="nmean")
                    with tc.high_priority():
                        for c in range(2):
                            nc.vector.tensor_mul(nmean[:, c, :], nsum[:, c, :],
                                                 inv_sb[:, ts])
                    # summary = pgat @ Wp.T + nmean @ Wn.T + (b_p + b_n)
                    sT = workp.tile([128, 2, NT], F32R, tag="sT", name="sT")
                    with tc.high_priority():
                        for oc in range(2):
                            ps = psp.tile([128, NT], F32, tag="sum", name="psS")
                            for hc in range(2):
                                nc.tensor.matmul(ps[:],
                                                 wp_sb[:, hc, oc * 128:(oc + 1) * 128],
                                                 pgat[:, hc, :],
                                                 start=(hc == 0), stop=False)
                            for hc in range(2):
                                nc.tensor.matmul(ps[:],
                                                 wn_sb[:, hc, oc * 128:(oc + 1) * 128],
                                                 nmean[:, hc, :],
                                                 start=False, stop=(hc == 1))
                            nc.vector.tensor_scalar_add(sT[:, oc, :], ps[:],
                                                        bias_sb[:, 2 + oc:3 + oc])
                    # GRU gates, per output chunk
                    for oc in range(2):
                        rp = psgp.tile([128, NT], F32, tag="gate", name="rp")
                        zp = psgp.tile([128, NT], F32, tag="gate", name="zp")
                        ip = psgp.tile([128, NT], F32, tag="gate", name="ip")
                        hp = psgp.tile([128, NT], F32, tag="gate", name="hp")
                        for gate, pst in ((0, rp), (1, zp)):
                            o0 = gate * H + oc * 128
                            for hc in range(2):
                                nc.tensor.matmul(pst[:], wih_sb[:, hc, o0:o0 + 128],
                                                 xf_in[:, hc, ts],
                                                 start=(hc == 0), stop=False)
                            for hc in range(2):
                                nc.tensor.matmul(pst[:], whh_sb[:, hc, o0:o0 + 128],
                                                 sT[:, hc, :],
                                                 start=False, stop=(hc == 1))
                        o0 = 2 * H + oc * 128
                        for hc in range(2):
                            nc.tensor.matmul(ip[:], wih_sb[:, hc, o0:o0 + 128],
                                             xf_in[:, hc, ts],
                                             start=(hc == 0), stop=(hc == 1))
                        for hc in range(2):
                            nc.tensor.matmul(hp[:], whh_sb[:, hc, o0:o0 + 128],
                                             sT[:, hc, :],
                                             start=(hc == 0), stop=(hc == 1))
                        r = tmpp.tile([128, NT], F32, tag="r", name="r")
                        nc.scalar.activation(r[:], rp[:], SIG,
                                             bias=bias_sb[:, 4 + oc:5 + oc])
                        z = tmpp.tile([128, NT], F32, tag="z", name="z")
                        nc.scalar.activation(z[:], zp[:], SIG,
                                             bias=bias_sb[:, 6 + oc:7 + oc])
                        # n = tanh((i_n + b_ih_n) + r * (h_n + b_hh_n))
                        hnr = tmpp.tile([128, NT], F32, tag="hnr", name="hnr")
                        nc.vector.scalar_tensor_tensor(
                            hnr[:], hp[:], bias_sb[:, 10 + oc:11 + oc], r[:],
                            op0=ADD, op1=MULT)
                        npre = tmpp.tile([128, NT], F32, tag="npre", name="npre")
                        nc.vector.scalar_tensor_tensor(
                            npre[:], ip[:], bias_sb[:, 8 + oc:9 + oc], hnr[:],
                            op0=ADD, op1=ADD)
                        nt_ = tmpp.tile([128, NT], F32, tag="nt", name="nt")
                        nc.scalar.activation(nt_[:], npre[:], TANH)
                        # x_new = n + z * (summary - n)
                        d = tmpp.tile([128, NT], F32, tag="d", name="d")
                        nc.vector.tensor_sub(d[:], sT[:, oc, :].bitcast(F32), nt_[:])
                        dz = tmpp.tile([128, NT], F32, tag="dz", name="dz")
                        nc.vector.tensor_mul(dz[:], d[:], z[:])
                        nc.vector.tensor_add(xf_out[:, oc, ts], dz[:], nt_[:])
                    if last:
                        nc.sync.dma_start(y.ap()[:, :, ts],
                                          xf_out[:, :, ts].bitcast(F32))
                    else:
                        write_table_tile(xf_out, t)
                if not last:
                    with tc.tile_wait_until(LBASE + T * 0.1 + 0.05):
                        ag_chunk(T - 1, xtab_out)
                cur = 1 - cur

    nc.compile()
    return nc


def _get_nc(depth: int):
    if depth not in _CACHE:
        _CACHE[depth] = _build(depth)
    return _CACHE[depth]


def _idx_layout(lin):
    """linear int16 idx list (len % 16 == 0) -> [128, len//16] wrapped in 16
    partitions, replicated across the 8 gpsimd core groups."""
    v = lin.reshape(-1, 16).T.astype(np.int16)        # [16, len//16]
    return np.tile(v, (8, 1))                         # [128, len//16]


def _chunk2(w):
    """[256, M] -> [128, 2, M] with [p, c, m] = w[c*128+p, m]."""
    M = w.shape[1]
    return np.ascontiguousarray(w.reshape(2, 128, M).transpose(1, 0, 2))


def prepare_inputs(inputs):
    """host-side preprocessing: returns in_maps for the 8 cores."""
    adj = np.asarray(inputs["nodeAdjacencySpecTensor"]).astype(np.int64)
    names = np.asarray(inputs["nodeNamesEncoded"], dtype=np.float32)
    attrs = np.asarray(inputs["nodeAttributesEncoded"], dtype=np.float32)

    parent = adj[:, 0]
    parent = np.clip(np.where(parent < 0, parent + N, parent), 0, N - 1)
    nbr = adj[:, 1:]
    mask = nbr >= 0
    cnt = np.maximum(mask.sum(1), 1).astype(np.float32)
    safe = np.where(mask, np.clip(nbr, 0, N - 1), ZROW).astype(np.int64)

    # remap global node id -> chunk-major xtab row (ZROW maps to itself)
    g = np.arange(N, dtype=np.int64)
    c_of = g // NC_REAL
    r_of = g % NC_REAL
    t_of = np.minimum(r_of // NT, T - 1)
    rows_t = np.asarray(TILE_ROWS, np.int64)[t_of]
    off_t = np.asarray(CHUNK_OFF[:T], np.int64)[t_of]
    remap = np.concatenate([off_t + c_of * rows_t + (r_of - t_of * NT),
                            np.array([ZROW], np.int64)])
    parent = remap[parent]
    safe = remap[safe]
    inv = (1.0 / cnt).astype(np.float32)

    feat = np.concatenate([names, attrs], axis=1)      # [N, 256] f32

    W_in = np.asarray(inputs["W_in"], np.float32)
    W_p = np.asarray(inputs["W_parent"], np.float32)
    W_n = np.asarray(inputs["W_neighbor"], np.float32)
    W_ih = np.asarray(inputs["W_ih"], np.float32)
    W_hh = np.asarray(inputs["W_hh"], np.float32)
    b_in = np.asarray(inputs["b_in"], np.float32)
    b_p = np.asarray(inputs["b_parent"], np.float32)
    b_n = np.asarray(inputs["b_neighbor"], np.float32)
    b_ih = np.asarray(inputs["b_ih"], np.float32)
    b_hh = np.asarray(inputs["b_hh"], np.float32)

    w_in_a = _chunk2(W_in.T)                            # [128, 2, 256]
    w_ih_a = _chunk2(W_ih.T)                            # [128, 2, 768]
    w_hh_a = _chunk2(W_hh.T)
    w_p_a = _chunk2(W_p.T).astype(BF)
    w_n_a = _chunk2(W_n.T).astype(BF)

    bias = np.zeros((128, 12), np.float32)
    for col, vec in ((0, b_in), (2, b_p + b_n), (4, (b_ih + b_hh)[0:H]),
                     (6, (b_ih + b_hh)[H:2 * H]), (8, b_ih[2 * H:3 * H]),
                     (10, b_hh[2 * H:3 * H])):
        bias[:, col] = vec[0:128]
        bias[:, col + 1] = vec[128:256]

    ident_b = np.eye(128, dtype=BF)

    shared = dict(w_in=w_in_a, w_ih=w_ih_a, w_hh=w_hh_a, w_p=w_p_a, w_n=w_n_a,
                  biases=bias, ident_b=ident_b)

    in_maps = []
    for c in range(NCORES):
        g0 = c * NC_REAL
        # features, transposed + padded
        f = np.zeros((NCP, 2 * 128), np.float32)
        f[:NC_REAL] = feat[g0:g0 + NC_REAL]
        featT_c = np.ascontiguousarray(
            f.T.reshape(2, 128, NCP).transpose(1, 0, 2))
        # inv count broadcast
        iv = np.ones(NCP, np.float32)
        iv[:NC_REAL] = inv[g0:g0 + NC_REAL]
        inv_c = np.broadcast_to(iv, (128, NCP)).copy()
        # indices
        par = np.full(NCP, ZROW, np.int64)
        par[:NC_REAL] = parent[g0:g0 + NC_REAL]
        nbrs = np.full((NCP, K), ZROW, np.int64)
        nbrs[:NC_REAL] = safe[g0:g0 + NC_REAL]
        gidx_t = np.zeros((128, T, NT + NT // 16), np.int16)
        for t in range(T):
            lin = np.concatenate([nbrs[t * NT:(t + 1) * NT].reshape(-1),
                                  par[t * NT:(t + 1) * NT]])
            gidx_t[:, t, :] = _idx_layout(lin)
        in_maps.append(dict(featT=featT_c, invcnt=inv_c, gidx=gidx_t,
                            **shared))
    return in_maps


def run(inputs, trace=False, **kw):
    depth = int(np.asarray(inputs["depth"]))
    nc = _get_nc(depth)
    in_maps = prepare_inputs(inputs)
    res = bass_utils.run_bass_kernel_spmd(nc, in_maps,
                                          core_ids=list(range(NCORES)),
                                          trace=trace, **kw)
    outs = []
    for c in range(NCORES):
        yt = np.asarray(res.results[c]["y"])          # [128, 2, NCP]
        xc = yt.transpose(1, 0, 2).reshape(2 * 128, NCP)   # [256, NCP]
        outs.append(xc[:, :NC_REAL].T)                # [NC_REAL, 256]
    out = np.concatenate(outs, axis=0)
    return np.ascontiguousarray(out.astype(np.float32)), res


def kernel(**inputs) -> np.ndarray:
    out, _ = run(inputs, trace=False)
    return out


# revision 15
# speedup vs baseline: 1.0477x; 1.0307x over previous
"""Trainium2 Bass kernel for nn_NodeInfoPropagate (GNN message passing).

Strategy (8 NeuronCores, node-parallel), v2 — pipelined:
  - Shard the 20000 nodes across 8 cores (2500/core, padded to 2560 = 5 tiles
    of 512).  Weights replicated.
  - Activations live on-chip transposed [feature-on-partition, node-on-free];
    all matmuls chain with zero transposes.  fp32 GRU path uses float32r.
  - Per layer the full x table [20000(+pad), 256] bf16 lives in each core's
    HBM; parent + neighbor rows are fetched with dma_gather(transpose=True)
    (SWDGE): cost is ~8ns/idx of GpSimd time, which is the critical resource.
    gather commutes with the linear maps, so only x is gathered and
    summary = x[par] @ Wp.T + mean_k x[nbr] @ Wn.T.
  - v2 overlap fixes vs v1:
      * TWO ping-pong x tables: the AllGather for layer l+1 streams into the
        other buffer while layer l is still gathering from the current one.
      * AllGather is chunked per 512-node tile and issued on the SYNC queue,
        so collectives never block the GpSimd gather queue and overlap with
        tile compute.  Only the last chunk's latency is exposed.
      * outputs written untransposed ([feat, node] f32); host reassembles.
      * per-tile feature loads for layer 0 (no big resident feat buffer).
"""

import sys

sys.path.insert(0, "/opt/trn_rl_repo")

import numpy as np
import ml_dtypes

import concourse.bass as bass
import concourse.bacc as bacc
import concourse.tile as tile
import concourse.mybir as mybir
from concourse import bass_utils

N = 20000
K = 16
H = 256
NCORES = 8
NC_REAL = N // NCORES          # 2500 real nodes per core
NT = 512                       # node tile (matmul free dim / PSUM bank)
T = 5                          # tiles per core
NCP = NT * T                   # 2560 padded nodes per core
ZROW = N                       # all-zero table row for invalid neighbors
NTAB = N + 128                 # table rows
NHALF = (NT // 2) * K          # 4096 neighbor idxs per half-tile
TILE_ROWS = [NT] * (T - 1) + [NC_REAL - NT * (T - 1)]  # [512,512,512,512,452]
# chunk-major table layout: chunk t holds all 8 cores' tile-t rows
# contiguously, so each AllGather chunk writes a contiguous range.
CHUNK_OFF = [0]
for _r in TILE_ROWS:
    CHUNK_OFF.append(CHUNK_OFF[-1] + NCORES * _r)  # [0,4096,...,16384,20000]

F32 = mybir.dt.float32
F32R = mybir.dt.float32r
BF16 = mybir.dt.bfloat16
I16 = mybir.dt.int16
BF = ml_dtypes.bfloat16

_CACHE = {}


def _build(depth: int):
    nc = bacc.Bacc("TRN2", target_bir_lowering=False, debug=False,
                   num_devices=NCORES)

    featT = nc.dram_tensor("featT", [128, 2, NCP], F32, kind="ExternalInput")
    invcnt = nc.dram_tensor("invcnt", [128, NCP], BF16, kind="ExternalInput")
    # per tile: 512 cols of wrapped neighbor idxs (8192), then 32 cols of
    # wrapped parent idxs (512) -- gathered as two instructions (8192 is the
    # max num_idxs a single SWDGE gather supports)
    gidx = nc.dram_tensor("gidx", [128, T, NT + NT // 16], I16, kind="ExternalInput")
    w_in = nc.dram_tensor("w_in", [128, 2, H], F32, kind="ExternalInput")
    w_ih = nc.dram_tensor("w_ih", [128, 2, 3 * H], F32, kind="ExternalInput")
    w_hh = nc.dram_tensor("w_hh", [128, 2, 3 * H], F32, kind="ExternalInput")
    w_p = nc.dram_tensor("w_p", [128, 2, H], BF16, kind="ExternalInput")
    w_n = nc.dram_tensor("w_n", [128, 2, H], BF16, kind="ExternalInput")
    # bias columns: 0-1 b_in, 2-3 b_p+b_n, 4-5 b_r, 6-7 b_z, 8-9 b_ih_n,
    # 10-11 b_hh_n  (per 128-feature chunk)
    biases = nc.dram_tensor("biases", [128, 12], F32, kind="ExternalInput")
    ident_b = nc.dram_tensor("ident_b", [128, 128], BF16, kind="ExternalInput")
    # output transposed: y[p, ch, i] = x_out[node i][ch*128 + p]
    y = nc.dram_tensor("y", [128, 2, NCP], F32, kind="ExternalOutput")

    SIG = mybir.ActivationFunctionType.Sigmoid
    TANH = mybir.ActivationFunctionType.Tanh
    ADD = mybir.AluOpType.add
    MULT = mybir.AluOpType.mult

    with tile.TileContext(nc) as tc:
        with (
            tc.tile_pool(name="const", bufs=1) as constp,
            tc.tile_pool(name="state", bufs=1) as statep,
            tc.tile_pool(name="dram", bufs=1, space="DRAM") as dramp,
            tc.tile_pool(name="feat", bufs=1) as featp,
            tc.tile_pool(name="gath", bufs=5) as gathp,
            tc.tile_pool(name="pgath", bufs=4) as pgathp,
            tc.tile_pool(name="work", bufs=2) as workp,
            tc.tile_pool(name="tmp", bufs=2) as tmpp,
            tc.tile_pool(name="ps", bufs=2, space="PSUM") as psp,
            tc.tile_pool(name="psg", bufs=6, space="PSUM") as psgp,
        ):
            # ---- resident constants -------------------------------------
            win_sb = constp.tile([128, 2, H], F32R, name="win_sb")
            nc.sync.dma_start(win_sb[:], w_in.ap().bitcast(F32R))
            wih_sb = constp.tile([128, 2, 3 * H], F32R, name="wih_sb")
            nc.sync.dma_start(wih_sb[:], w_ih.ap().bitcast(F32R))
            whh_sb = constp.tile([128, 2, 3 * H], F32R, name="whh_sb")
            nc.sync.dma_start(whh_sb[:], w_hh.ap().bitcast(F32R))
            wp_sb = constp.tile([128, 2, H], BF16, name="wp_sb")
            nc.sync.dma_start(wp_sb[:], w_p.ap())
            wn_sb = constp.tile([128, 2, H], BF16, name="wn_sb")
            nc.sync.dma_start(wn_sb[:], w_n.ap())
            bias_sb = constp.tile([128, 12], F32, name="bias_sb")
            nc.sync.dma_start(bias_sb[:], biases.ap())
            idb_sb = constp.tile([128, 128], BF16, name="idb_sb")
            nc.sync.dma_start(idb_sb[:], ident_b.ap())
            inv_sb = constp.tile([128, NCP], BF16, name="inv_sb")
            nc.sync.dma_start(inv_sb[:], invcnt.ap())
            gidx_sb = constp.tile([128, T, NT + NT // 16], I16, name="gidx_sb")
            nc.sync.dma_start(gidx_sb[:], gidx.ap())

            xF = [statep.tile([128, 2, NCP], F32R, name=f"xF{i}") for i in range(2)]

            xloc = dramp.tile([NCP, H], BF16, name="xloc")
            xtabs = [dramp.tile([NTAB, H], BF16, name=f"xtab{i}") for i in range(2)]

            # zero row for invalid-neighbor gathers (both tables, once)
            zero_sb = constp.tile([128, H], BF16, name="zero_sb")
            nc.vector.memset(zero_sb[:], 0.0)
            for xt in xtabs:
                nc.sync.dma_start(xt[ZROW:ZROW + 1, :], zero_sb[0:1, :])

            def write_table_tile(xf, t):
                """cast tile t of xf to bf16, transpose to row-major, DMA to
                xloc rows."""
                ts = slice(t * NT, (t + 1) * NT)
                xb = workp.tile([128, 2, NT], BF16, tag="xb", name="xb")
                nc.vector.tensor_copy(xb[:], xf[:, :, ts].bitcast(F32))
                for b in range(NT // 128):
                    rm = workp.tile([128, 2, 128], BF16, tag="rm", name="rm")
                    for c in range(2):
                        pst = psp.tile([128, 128], BF16, tag="sum", name="pst")
                        nc.tensor.transpose(pst[:], xb[:, c, b * 128:(b + 1) * 128],
                                            idb_sb[:])
                        nc.vector.tensor_copy(rm[:, c, :], pst[:])
                    r0 = t * NT + b * 128
                    nc.sync.dma_start(xloc[r0:r0 + 128, :], rm[:])

            def ag_chunk(t, xtab_dst):
                """AllGather tile-t rows of xloc into the strided per-core
                slots of xtab_dst.  Issued on the SYNC queue so it never
                blocks the gather (GpSimd) queue."""
                rows = TILE_ROWS[t]
                r0 = t * NT
                ins_ap = xloc[r0:r0 + rows, :]
                o0 = CHUNK_OFF[t]
                outs_ap = xtab_dst[o0:o0 + NCORES * rows, :]
                with tc.high_priority():
                    bass.BassGpSimd.collective_compute(
                        nc.gpsimd,
                        "AllGather", mybir.AluOpType.bypass,
                        replica_groups=[list(range(NCORES))],
                        ins=[ins_ap.opt()],
                        outs=[outs_ap.opt()],
                    )

            # ---- layer 0: x0 = W_in @ feat + b_in ------------------------
            for t in range(T):
                ts = slice(t * NT, (t + 1) * NT)
                ft = featp.tile([128, 2, NT], F32R, tag="ft", name="ft")
                nc.sync.dma_start(ft[:], featT.ap().bitcast(F32R)[:, :, ts])
                for oc in range(2):
                    ps = psp.tile([128, NT], F32, tag="sum", name="ps0")
                    for dc in range(2):
                        nc.tensor.matmul(ps[:], win_sb[:, dc, oc * 128:(oc + 1) * 128],
                                         ft[:, dc, :], start=(dc == 0), stop=(dc == 1))
                    nc.vector.tensor_scalar_add(xF[0][:, oc, ts], ps[:],
                                                bias_sb[:, oc:oc + 1])
                if depth == 0:
                    nc.sync.dma_start(y.ap()[:, :, ts],
                                      xF[0][:, :, ts].bitcast(F32))
                else:
                    write_table_tile(xF[0], t)
                    ag_chunk(t, xtabs[0])

            # ---- GRU layers ---------------------------------------------
            cur = 0
            for layer in range(depth):
                LBASE = 1.0 + layer * 1.0
                last = layer == depth - 1
                xf_in, xf_out = xF[cur], xF[1 - cur]
                xtab_in = xtabs[layer % 2]
                xtab_out = xtabs[(layer + 1) % 2]
                for t in range(T):
                    ts = slice(t * NT, (t + 1) * NT)
                    # transpose-gathers cap at 4096 idxs: two neighbor halves
                    pg = pgathp.tile([128, 2, NT], BF16, tag="pgat", name="pg")
                    with tc.tile_wait_until(LBASE + t * 0.1):
                        nc.gpsimd.dma_gather(pg[:], xtab_in[:],
                                             gidx_sb[:, t, NT:],
                                             NT, NT, H, transpose=True,
                                             single_packet=False)
                    pgat = pg[:, :, :]
                    nsum = workp.tile([128, 2, NT], F32, tag="nsum", name="nsum")
                    for hf in range(2):
                        hs = slice(hf * (NT // 2), (hf + 1) * (NT // 2))
                        ngat = gathp.tile([128, 2, NHALF], BF16, tag="ngat",
                                          name="ngat")
                        with tc.tile_wait_until(LBASE + t * 0.1 + 0.02 + hf * 0.03):
                            nc.gpsimd.dma_gather(
                                ngat[:], xtab_in[:],
                                gidx_sb[:, t, hf * (NT // 2):(hf + 1) * (NT // 2)],
                                NHALF, NHALF, H, transpose=True,
                                single_packet=False)
                        with tc.high_priority():
                            for c in range(2):
                                nc.vector.tensor_reduce(
                                    nsum[:, c, hs],
                                    ngat[:, c, :].rearrange("p (n k) -> p n k", k=K),
                                    axis=mybir.AxisListType.X, op=ADD)
                    if not last and t >= 1:
                        with tc.tile_wait_until(LBASE + t * 0.1 + 0.07):
                            ag_chunk(t - 1, xtab_out)
                    nmean = workp.tile([128, 2, NT], BF16, tag="nmean", name="nmean")
                    with tc.high_priority():
                        for c in range(2):
                            nc.vector.tensor_mul(nmean[:, c, :], nsum[:, c, :],
                                                 inv_sb[:, ts])
                    # summary = pgat @ Wp.T + nmean @ Wn.T + (b_p + b_n)
                    sT = workp.tile([128, 2, NT], F32R, tag="sT", name="sT")
                    with tc.high_priority():
                        for oc in range(2):
                            ps = psp.tile([128, NT], F32, tag="sum", name="psS")
                            for hc in range(2):
                                nc.tensor.matmul(ps[:],
                                                 wp_sb[:, hc, oc * 128:(oc + 1) * 128],
                                                 pgat[:, hc, :],
                                                 start=(hc == 0), stop=False)
                            for hc in range(2):
                                nc.tensor.matmul(ps[:],
                                                 wn_sb[:, hc, oc * 128:(oc + 1) * 128],
                                                 nmean[:, hc, :],
                                                 start=False, stop=(hc == 1))
                            nc.vector.tensor_scalar_add(sT[:, oc, :], ps[:],
                                                        bias_sb[:, 2 + oc:3 + oc])
                    # GRU gates, per output chunk
                    for oc in range(2):
                        rp = psgp.tile([128, NT], F32, tag="gate", name="rp")
                        zp = psgp.tile([128, NT], F32, tag="gate", name="zp")
                        ip = psgp.tile([128, NT], F32, tag="gate", name="ip")
                        hp = psgp.tile([128, NT], F32, tag="gate", name="hp")
                        for gate, pst in ((0, rp), (1, zp)):
                            o0 = gate * H + oc * 128
                            for hc in range(2):
                                nc.tensor.matmul(pst[:], wih_sb[:, hc, o0:o0 + 128],
                                                 xf_in[:, hc, ts],
                                                 start=(hc == 0), stop=False)
                            for hc in range(2):
                                nc.tensor.matmul(pst[:], whh_sb[:, hc, o0:o0 + 128],
                                                 sT[:, hc, :],
                                                 start=False, stop=(hc == 1))
                        o0 = 2 * H + oc * 128
                        for hc in range(2):
                            nc.tensor.matmul(ip[:], wih_sb[:, hc, o0:o0 + 128],
                                             xf_in[:, hc, ts],
                                             start=(hc == 0), stop=(hc == 1))
                        for hc in range(2):
                            nc.tensor.matmul(hp[:], whh_sb[:, hc, o0:o0 + 128],
                                             sT[:, hc, :],
                                             start=(hc == 0), stop=(hc == 1))
                        r = tmpp.tile([128, NT], F32, tag="r", name="r")
                        nc.scalar.activation(r[:], rp[:], SIG,
                                             bias=bias_sb[:, 4 + oc:5 + oc])
                        z = tmpp.tile([128, NT], F32, tag="z", name="z")
                        nc.scalar.activation(z[:], zp[:], SIG,
                                             bias=bias_sb[:, 6 + oc:7 + oc])
                        # n = tanh((i_n + b_ih_n) + r * (h_n + b_hh_n))
                        hnr = tmpp.tile([128, NT], F32, tag="tA", name="hnr")
                        nc.vector.scalar_tensor_tensor(
                            hnr[:], hp[:], bias_sb[:, 10 + oc:11 + oc], r[:],
                            op0=ADD, op1=MULT)
                        npre = tmpp.tile([128, NT], F32, tag="tB", name="npre")
                        nc.vector.scalar_tensor_tensor(
                            npre[:], ip[:], bias_sb[:, 8 + oc:9 + oc], hnr[:],
                            op0=ADD, op1=ADD)
                        nt_ = tmpp.tile([128, NT], F32, tag="nt", name="nt")
                        nc.scalar.activation(nt_[:], npre[:], TANH)
                        # x_new = n + z * (summary - n)
                        d = tmpp.tile([128, NT], F32, tag="tA", name="d")
                        nc.vector.tensor_sub(d[:], sT[:, oc, :].bitcast(F32), nt_[:])
                        dz = tmpp.tile([128, NT], F32, tag="tB", name="dz")
                        nc.vector.tensor_mul(dz[:], d[:], z[:])
                        nc.vector.tensor_add(xf_out[:, oc, ts], dz[:], nt_[:])
                    if last:
                        nc.sync.dma_start(y.ap()[:, :, ts],
                                          xf_out[:, :, ts].bitcast(F32))
                    else:
                        write_table_tile(xf_out, t)
                if not last:
                    with tc.tile_wait_until(LBASE + T * 0.1 + 0.05):
                        ag_chunk(T - 1, xtab_out)
                cur = 1 - cur

    nc.compile()
    return nc


def _get_nc(depth: int):
    if depth not in _CACHE:
        _CACHE[depth] = _build(depth)
    return _CACHE[depth]


def _idx_layout(lin):
    """linear int16 idx list (len % 16 == 0) -> [128, len//16] wrapped in 16
    partitions, replicated across the 8 gpsimd core groups."""
    v = lin.reshape(-1, 16).T.astype(np.int16)        # [16, len//16]
    return np.tile(v, (8, 1))                         # [128, len//16]


def _chunk2(w):
    """[256, M] -> [128, 2, M] with [p, c, m] = w[c*128+p, m]."""
    M = w.shape[1]
    return np.ascontiguousarray(w.reshape(2, 128, M).transpose(1, 0, 2))


def prepare_inputs(inputs):
    """host-side preprocessing: returns in_maps for the 8 cores."""
    adj = np.asarray(inputs["nodeAdjacencySpecTensor"]).astype(np.int64)
    names = np.asarray(inputs["nodeNamesEncoded"], dtype=np.float32)
    attrs = np.asarray(inputs["nodeAttributesEncoded"], dtype=np.float32)

    parent = adj[:, 0]
    parent = np.clip(np.where(parent < 0, parent + N, parent), 0, N - 1)
    nbr = adj[:, 1:]
    mask = nbr >= 0
    cnt = np.maximum(mask.sum(1), 1).astype(np.float32)
    safe = np.where(mask, np.clip(nbr, 0, N - 1), ZROW).astype(np.int64)

    # remap global node id -> chunk-major xtab row (ZROW maps to itself)
    g = np.arange(N, dtype=np.int64)
    c_of = g // NC_REAL
    r_of = g % NC_REAL
    t_of = np.minimum(r_of // NT, T - 1)
    rows_t = np.asarray(TILE_ROWS, np.int64)[t_of]
    off_t = np.asarray(CHUNK_OFF[:T], np.int64)[t_of]
    remap = np.concatenate([off_t + c_of * rows_t + (r_of - t_of * NT),
                            np.array([ZROW], np.int64)])
    parent = remap[parent]
    safe = remap[safe]
    inv = (1.0 / cnt).astype(np.float32)

    feat = np.concatenate([names, attrs], axis=1)      # [N, 256] f32

    W_in = np.asarray(inputs["W_in"], np.float32)
    W_p = np.asarray(inputs["W_parent"], np.float32)
    W_n = np.asarray(inputs["W_neighbor"], np.float32)
    W_ih = np.asarray(inputs["W_ih"], np.float32)
    W_hh = np.asarray(inputs["W_hh"], np.float32)
    b_in = np.asarray(inputs["b_in"], np.float32)
    b_p = np.asarray(inputs["b_parent"], np.float32)
    b_n = np.asarray(inputs["b_neighbor"], np.float32)
    b_ih = np.asarray(inputs["b_ih"], np.float32)
    b_hh = np.asarray(inputs["b_hh"], np.float32)

    w_in_a = _chunk2(W_in.T)                            # [128, 2, 256]
    w_ih_a = _chunk2(W_ih.T)                            # [128, 2, 768]
    w_hh_a = _chunk2(W_hh.T)
    w_p_a = _chunk2(W_p.T).astype(BF)
    w_n_a = _chunk2(W_n.T).astype(BF)

    bias = np.zeros((128, 12), np.float32)
    for col, vec in ((0, b_in), (2, b_p + b_n), (4, (b_ih + b_hh)[0:H]),
                     (6, (b_ih + b_hh)[H:2 * H]), (8, b_ih[2 * H:3 * H]),
                     (10, b_hh[2 * H:3 * H])):
        bias[:, col] = vec[0:128]
        bias[:, col + 1] = vec[128:256]

    ident_b = np.eye(128, dtype=BF)

    shared = dict(w_in=w_in_a, w_ih=w_ih_a, w_hh=w_hh_a, w_p=w_p_a, w_n=w_n_a,
                  biases=bias, ident_b=ident_b)

    in_maps = []
    for c in range(NCORES):
        g0 = c * NC_REAL
        # features, transposed + padded
        f = np.zeros((NCP, 2 * 128), np.float32)
        f[:NC_REAL] = feat[g0:g0 + NC_REAL]
        featT_c = np.ascontiguousarray(
            f.T.reshape(2, 128, NCP).transpose(1, 0, 2))
        # inv count broadcast
        iv = np.ones(NCP, np.float32)
        iv[:NC_REAL] = inv[g0:g0 + NC_REAL]
        inv_c = np.broadcast_to(iv, (128, NCP)).astype(BF).copy()
        # indices
        par = np.full(NCP, ZROW, np.int64)
        par[:NC_REAL] = parent[g0:g0 + NC_REAL]
        nbrs = np.full((NCP, K), ZROW, np.int64)
        nbrs[:NC_REAL] = safe[g0:g0 + NC_REAL]
        gidx_t = np.zeros((128, T, NT + NT // 16), np.int16)
        for t in range(T):
            lin = np.concatenate([nbrs[t * NT:(t + 1) * NT].reshape(-1),
                                  par[t * NT:(t + 1) * NT]])
            gidx_t[:, t, :] = _idx_layout(lin)
        in_maps.append(dict(featT=featT_c, invcnt=inv_c, gidx=gidx_t,
                            **shared))
    return in_maps


def run(inputs, trace=False, **kw):
    depth = int(np.asarray(inputs["depth"]))
    nc = _get_nc(depth)
    in_maps = prepare_inputs(inputs)
    res = bass_utils.run_bass_kernel_spmd(nc, in_maps,
                                          core_ids=list(range(NCORES)),
                                          trace=trace, **kw)
    outs = []
    for c in range(NCORES):
        yt = np.asarray(res.results[c]["y"])          # [128, 2, NCP]
        xc = yt.transpose(1, 0, 2).reshape(2 * 128, NCP)   # [256, NCP]
        outs.append(xc[:, :NC_REAL].T)                # [NC_REAL, 256]
    out = np.concatenate(outs, axis=0)
    return np.ascontiguousarray(out.astype(np.float32)), res


def kernel(**inputs) -> np.ndarray:
    out, _ = run(inputs, trace=False)
    return out
